# revision 27
# baseline (speedup 1.0000x reference)
"""MoE (DbrxExperts) expert-parallel Trainium2 kernel.

Strategy (v2):
  - Host: compute per-(expert,token) combine weights cw, gather each
    expert's routed tokens exactly (no common-C padding), pre-transpose
    operands, fp16 everywhere (rel err ~6e-4 vs 2e-2 budget).
  - Expert->core assignment: the 8 largest experts form "slot A" (one
    per core), the 8 smallest form "slot B".  The SPMD program is built
    for (n1, n2) = (max A count, max B count); this minimizes the
    uniform per-core token count  n1+n2  (4096 -> ~3785 on typical
    routing), which is what the PE stream time scales with.
  - Device (8 cores, SPMD, 2 experts/core): per expert
        gate_T = W1T_blocks^T @ XT     [F, C]   (contract H)
        up_T   = V1T_blocks^T @ XT     [F, C]
        hact_T = silu(gate_T) * up_T   [F, C]   (ACT + DVE, fp16)
        down   = hact_T_blocks^T @ W2  [C, H]   (contract F)
    PSUM fp32, output y fp32.
  - Head optimizations: first f-tile weights DMA'd before the x chunk,
    and a short burst of dummy matmuls warms the PE HAM clock gate
    while the first DMAs land.
  - Host: out[tokens_e] += down_e * cw_e.
"""

import numpy as np
from contextlib import ExitStack

N_CORES = 8
B, S, H = 4, 2048, 1024
F, E = 2048, 16
T = B * S
E_LOC = E // N_CORES  # 2 experts per core (slot A + slot B)

P = 128
HT = H // P   # 8  h-tiles
FT = F // P   # 16 f-tiles
CH = 1024     # max token-chunk width

TRACE = False          # test.py sets this for profiled runs
TRACE_CORES = [7]      # core-0 NTFF capture crashes fast kernels here
MM_DTYPE = "fp16"      # "fp16" | "bf16" | "fp32r"
WARM_MMS = 28          # dummy matmuls to release the HAM clock gate
LAST_RESULT = None     # BassKernelResults of last run (for test.py)

_nc_cache = {}


def _chunks(n):
    """Token chunks [512, CH, ..., remainder]: the first chunk is small so
    its x tiles land at DMA rate without stalling the first f-tiles."""
    out = []
    c0 = 0
    if n > 512:
        out.append((0, 512))
        c0 = 512
    while n - c0 > CH:
        out.append((c0, CH))
        c0 += CH
    out.append((c0, n - c0))
    return out


def _parts(S_):
    """Split a chunk into <=512-wide matmul parts."""
    out = []
    o = 0
    while S_ - o > 512:
        out.append((o, 512))
        o += 512
    out.append((o, S_ - o))
    return out


def _build_nc(n1, n2):
    # NOTE: reads module-global MM_DTYPE
    import concourse.tile as tile
    from concourse import bacc, mybir

    nc = bacc.Bacc("TRN2", target_bir_lowering=False, debug=False,
                   enable_asserts=False, num_devices=N_CORES)
    dt = mybir.dt.float32
    mdt = {"fp16": mybir.dt.float16, "bf16": mybir.dt.bfloat16,
           "fp32r": mybir.dt.float32r}[MM_DTYPE]
    SILU = mybir.ActivationFunctionType.Silu
    Ctot = n1 + n2

    # xt blocked [p(h%128), o(h//128), c] so a 2-h-tile SBUF tile is a
    # plain slice
    xt = nc.dram_tensor("xt", [P, HT, Ctot], mdt, kind="ExternalInput").ap()
    # w1t/v1t pre-blocked: [e, ft, p(h%128), o(h//128), f] so each (e, ft)
    # slice is contiguous and DMAs as 128 x 2KB descriptors
    w1t = nc.dram_tensor("w1t", [E_LOC, FT, P, HT, P], mdt,
                         kind="ExternalInput").ap()
    v1t = nc.dram_tensor("v1t", [E_LOC, FT, P, HT, P], mdt,
                         kind="ExternalInput").ap()
    w2 = nc.dram_tensor("w2", [E_LOC, F, H], mdt, kind="ExternalInput").ap()
    # y transposed [H, Ctot]: GEMM3 keeps w2 stationary (output partition
    # = h-tile), so token columns need no 128-padding
    y = nc.dram_tensor("y", [H, Ctot], dt, kind="ExternalOutput").ap()

    with tile.TileContext(nc) as tc:
        with ExitStack() as ctx:
            xt_pool = ctx.enter_context(tc.tile_pool(name="xt", bufs=HT))
            wst_pool = ctx.enter_context(tc.tile_pool(name="wst", bufs=4))
            w2_pool = ctx.enter_context(tc.tile_pool(name="w2sb", bufs=FT))
            hact_pool = ctx.enter_context(tc.tile_pool(name="hact",
                                                       bufs=2 * FT))
            silu_pool = ctx.enter_context(tc.tile_pool(name="silu", bufs=4))
            out_pool = ctx.enter_context(tc.tile_pool(name="out", bufs=4))
            warm_pool = ctx.enter_context(tc.tile_pool(name="warm", bufs=1))
            # 6 banks for GEMM1/2 g/u accumulators, 2 for GEMM3 so the
            # down accumulators never wait on the GLU drain
            ps_pool = ctx.enter_context(tc.tile_pool(name="ps", bufs=6,
                                                     space="PSUM"))
            psd_pool = ctx.enter_context(tc.tile_pool(name="psd", bufs=2,
                                                      space="PSUM"))

            # Dummy matmuls: PE activity during the DMA head releases the
            # HAM clock gate (4/8 -> 8/8) before the real stream starts.
            warm_sb = warm_pool.tile([P, P], mdt, tag="warm")
            nc.any.memset(warm_sb[:], 0)
            wps = psd_pool.tile([P, 512], dt, tag="psd", name="warm")
            for _ in range(WARM_MMS):
                nc.tensor.matmul(wps[:, :P], warm_sb[:], warm_sb[:],
                                 start=True, stop=True)

            for e in range(E_LOC):
                cnt = n1 if e == 0 else n2
                cbase = 0 if e == 0 else n1
                w2_sb = []

                for ci, (c0, S_) in enumerate(_chunks(cnt)):
                    parts = _parts(S_)

                    # DMA order w1(ft0), xt(ht0-1), v1(ft0), xt(rest).
                    # xt tiles carry two h-tiles per dma_start: descriptor
                    # issue costs ~640ns each on the sync queue, so fewer,
                    # larger issues shorten the head
                    w1s0 = wst_pool.tile([P, HT, P], mdt, tag="wst")
                    v1s0 = wst_pool.tile([P, HT, P], mdt, tag="wst")
                    nc.sync.dma_start(w1s0[:], w1t[e, 0])
                    xt_sb = []
                    for hp in range(HT // 2):
                        t = xt_pool.tile([P, 2, CH], mdt, tag="xt")
                        nc.sync.dma_start(
                            t[:, :, :S_],
                            xt[:, 2 * hp:2 * hp + 2,
                               cbase + c0:cbase + c0 + S_])
                        xt_sb.append(t)
                        if hp == 0:
                            nc.sync.dma_start(v1s0[:], v1t[e, 0])

                    # GEMM1/2 + GLU -> hact_T tiles [128, S_] per f-tile
                    hact_sb = []
                    for ft in range(FT):
                        if ft == 0:
                            w1s, v1s = w1s0, v1s0
                        else:
                            w1s = wst_pool.tile([P, HT, P], mdt, tag="wst")
                            v1s = wst_pool.tile([P, HT, P], mdt, tag="wst")
                            nc.sync.dma_start(w1s[:], w1t[e, ft])
                            nc.sync.dma_start(v1s[:], v1t[e, ft])
                        h_t = hact_pool.tile([P, CH], mdt, tag="hact")
                        g_tiles = [ps_pool.tile([P, 512], dt, tag="ps",
                                                name=f"g{i_}")
                                   for i_ in range(len(parts))]
                        u_tiles = [ps_pool.tile([P, 512], dt, tag="ps",
                                                name=f"u{i_}")
                                   for i_ in range(len(parts))]  # <=4 banks
                        for ht in range(HT):
                            xs = xt_sb[ht // 2]
                            for i_, (o_, p_) in enumerate(parts):
                                nc.tensor.matmul(
                                    g_tiles[i_][:, :p_], w1s[:, ht, :],
                                    xs[:, ht % 2, o_:o_ + p_],
                                    start=(ht == 0), stop=(ht == HT - 1))
                            for i_, (o_, p_) in enumerate(parts):
                                nc.tensor.matmul(
                                    u_tiles[i_][:, :p_], v1s[:, ht, :],
                                    xs[:, ht % 2, o_:o_ + p_],
                                    start=(ht == 0), stop=(ht == HT - 1))
                        for i_, (o_, p_) in enumerate(parts):
                            sl = silu_pool.tile([P, 512], mdt, tag="sl")
                            nc.scalar.activation(sl[:, :p_],
                                                 g_tiles[i_][:, :p_], SILU)
                            nc.vector.tensor_mul(
                                h_t[:, o_:o_ + p_], sl[:, :p_],
                                u_tiles[i_][:, :p_])
                        hact_sb.append(h_t)

                    if ci == 0:
                        for ft in range(FT):
                            t = w2_pool.tile([P, H], mdt, tag="w2",
                                             name=f"w2_{ft}")
                            nc.sync.dma_start(
                                t[:], w2[e, ft * P:(ft + 1) * P, :])
                            w2_sb.append(t)

                    # GEMM3: down^T[h, c] accumulated over f-tiles with w2
                    # stationary (LDW sources long-resident weights, token
                    # columns exact).  Two (h-tile, part) groups interleave
                    # so consecutive matmuls hit different PSUM banks.
                    groups = [(hht, o_, p_) for hht in range(HT)
                              for (o_, p_) in parts]
                    # in the very last chunk there is no following GEMM1/2
                    # work to hide the pair-boundary copy latency, so draw
                    # accumulators from the (by now idle) 6-buffer pool,
                    # and order pairs so the smallest parts finish last
                    last = (e == E_LOC - 1) and (c0 + S_ == cnt)
                    dpool, dtag = (ps_pool, "ps") if last else (psd_pool,
                                                                "psd")
                    if last:
                        groups.sort(key=lambda g: -g[2])
                    for gi in range(0, len(groups), 2):
                        ga, gb = groups[gi], groups[gi + 1]
                        da = dpool.tile([P, 512], dt, tag=dtag, name="da")
                        db = dpool.tile([P, 512], dt, tag=dtag, name="db")
                        for ft in range(FT):
                            for (hht, o_, p_), dd in ((ga, da), (gb, db)):
                                nc.tensor.matmul(
                                    dd[:, :p_],
                                    w2_sb[ft][:, hht * P:(hht + 1) * P],
                                    hact_sb[ft][:, o_:o_ + p_],
                                    start=(ft == 0), stop=(ft == FT - 1))
                        for (hht, o_, p_), dd in ((ga, da), (gb, db)):
                            o_t = out_pool.tile([P, 512], dt, tag="o")
                            nc.any.tensor_copy(o_t[:, :p_], dd[:, :p_])
                            nc.sync.dma_start(
                                y[hht * P:(hht + 1) * P,
                                  cbase + c0 + o_:cbase + c0 + o_ + p_],
                                o_t[:, :p_])
    nc.compile()
    return nc


def _get_nc(n1, n2):
    key = (n1, n2, MM_DTYPE)
    if key not in _nc_cache:
        _nc_cache[key] = _build_nc(n1, n2)
    return _nc_cache[key]


def prepare(x, top_weights, top_experts, w1, v1, w2):
    """Host-side routing + sharded input construction.
    Returns (n1, n2, in_maps, assign, idx, counts, cw)."""
    x = np.asarray(x, dtype=np.float32)
    top_weights = np.asarray(top_weights, dtype=np.float32)
    top_experts = np.asarray(top_experts).astype(np.int64)
    w1 = np.asarray(w1, dtype=np.float32)
    v1 = np.asarray(v1, dtype=np.float32)
    w2 = np.asarray(w2, dtype=np.float32)
    hdt = {"fp16": np.float16, "fp32r": np.float32}.get(MM_DTYPE)
    if hdt is None:
        import ml_dtypes
        hdt = ml_dtypes.bfloat16

    xf = x.reshape(T, H)

    # combine weights per (token, expert); duplicate slots sum
    cw = np.zeros((T, E), dtype=np.float32)
    np.add.at(cw, (np.arange(T)[:, None], top_experts), top_weights)

    idx = [np.nonzero(cw[:, e])[0] for e in range(E)]
    counts = np.array([len(i) for i in idx])

    # slot A = 8 largest experts, slot B = 8 smallest; program built for
    # the max count in each slot -> minimal uniform per-core token count
    order = np.argsort(-counts, kind="stable")
    slot_a, slot_b = order[:N_CORES], order[N_CORES:]
    n1 = max(128, int(counts[slot_a].max()))
    n2 = max(128, int(counts[slot_b].max()))
    assign = [(int(slot_a[m]), int(slot_b[m])) for m in range(N_CORES)]

    def _block(w, ids):
        # [e, F, H] -> [e, ft, p(h%128), o(h//128), f]: each (e, ft)
        # slice contiguous so the DMA runs 128 x 2KB descriptors
        wl = w[ids].reshape(E_LOC, FT, P, HT, P)  # [e, ft, f, o, p]
        return np.ascontiguousarray(
            wl.transpose(0, 1, 4, 3, 2)).astype(hdt)

    in_maps = []
    for m in range(N_CORES):
        ea, eb = assign[m]
        XT = np.zeros((H, n1 + n2), dtype=hdt)
        XT[:, :counts[ea]] = xf[idx[ea]].T.astype(hdt)
        XT[:, n1:n1 + counts[eb]] = xf[idx[eb]].T.astype(hdt)
        # blocked [p(h%128), o(h//128), c]
        XT = np.ascontiguousarray(
            XT.reshape(HT, P, n1 + n2).transpose(1, 0, 2))
        ids = [ea, eb]
        in_maps.append({
            "xt": XT,
            "w1t": _block(w1, ids),
            "v1t": _block(v1, ids),
            "w2": np.ascontiguousarray(w2[ids]).astype(hdt),
        })
    return n1, n2, in_maps, assign, idx, counts, cw


def combine(results, n1, assign, idx, counts, cw):
    """Weighted scatter-add of per-core expert outputs into [B, S, H]."""
    out = np.zeros((T, H), dtype=np.float32)
    for m in range(N_CORES):
        ym = results[m]["y"]  # [H, n1+n2]
        ea, eb = assign[m]
        out[idx[ea]] += ym[:, :counts[ea]].T * cw[idx[ea], ea][:, None]
        out[idx[eb]] += (ym[:, n1:n1 + counts[eb]].T
                         * cw[idx[eb], eb][:, None])
    return out.reshape(B, S, H)


def kernel(x, weights, top_weights, top_experts, w1, v1, w2):
    global LAST_RESULT
    n1, n2, in_maps, assign, idx, counts, cw = prepare(
        x, top_weights, top_experts, w1, v1, w2)
    nc = _get_nc(n1, n2)
    from concourse.bass_utils import run_bass_kernel_spmd
    res = run_bass_kernel_spmd(nc, in_maps, list(range(N_CORES)), trace=TRACE,
                               trace_cores=TRACE_CORES if TRACE else None)
    LAST_RESULT = res
    return combine(res.results, n1, assign, idx, counts, cw)


# revision 28
# speedup vs baseline: 1.0415x; 1.0415x over previous
"""MoE (DbrxExperts) expert-parallel Trainium2 kernel.

Strategy (v2):
  - Host: compute per-(expert,token) combine weights cw, gather each
    expert's routed tokens exactly (no common-C padding), pre-transpose
    operands, fp16 everywhere (rel err ~6e-4 vs 2e-2 budget).
  - Expert->core assignment: the 8 largest experts form "slot A" (one
    per core), the 8 smallest form "slot B".  The SPMD program is built
    for (n1, n2) = (max A count, max B count); this minimizes the
    uniform per-core token count  n1+n2  (4096 -> ~3785 on typical
    routing), which is what the PE stream time scales with.
  - Device (8 cores, SPMD, 2 experts/core): per expert
        gate_T = W1T_blocks^T @ XT     [F, C]   (contract H)
        up_T   = V1T_blocks^T @ XT     [F, C]
        hact_T = silu(gate_T) * up_T   [F, C]   (ACT + DVE, fp16)
        down   = hact_T_blocks^T @ W2  [C, H]   (contract F)
    PSUM fp32, output y fp32.
  - Head optimizations: first f-tile weights DMA'd before the x chunk,
    and a short burst of dummy matmuls warms the PE HAM clock gate
    while the first DMAs land.
  - Host: out[tokens_e] += down_e * cw_e.
"""

import numpy as np
from contextlib import ExitStack

N_CORES = 8
B, S, H = 4, 2048, 1024
F, E = 2048, 16
T = B * S
E_LOC = E // N_CORES  # 2 experts per core (slot A + slot B)

P = 128
HT = H // P   # 8  h-tiles
FT = F // P   # 16 f-tiles
CH = 1024     # max token-chunk width

TRACE = False          # test.py sets this for profiled runs
TRACE_CORES = [7]      # core-0 NTFF capture crashes fast kernels here
MM_DTYPE = "fp16"      # "fp16" | "bf16" | "fp32r"
WARM_MMS = 28          # dummy matmuls to release the HAM clock gate
LAST_RESULT = None     # BassKernelResults of last run (for test.py)

_nc_cache = {}


def _chunks(n):
    """Token chunks of <=CH: [CH, CH, ..., remainder].  (A 512-wide first
    chunk was tried for a faster head ramp but doubles the weight-DMA rate
    the PE demands mid-stream and causes starvation gaps.)"""
    out = []
    c0 = 0
    while n - c0 > CH:
        out.append((c0, CH))
        c0 += CH
    out.append((c0, n - c0))
    return out


def _parts(S_):
    """Split a chunk into <=512-wide matmul parts."""
    out = []
    o = 0
    while S_ - o > 512:
        out.append((o, 512))
        o += 512
    out.append((o, S_ - o))
    return out


def _build_nc(n1, n2):
    # NOTE: reads module-global MM_DTYPE
    import concourse.tile as tile
    from concourse import bacc, mybir

    nc = bacc.Bacc("TRN2", target_bir_lowering=False, debug=False,
                   enable_asserts=False, num_devices=N_CORES)
    dt = mybir.dt.float32
    mdt = {"fp16": mybir.dt.float16, "bf16": mybir.dt.bfloat16,
           "fp32r": mybir.dt.float32r}[MM_DTYPE]
    SILU = mybir.ActivationFunctionType.Silu
    Ctot = n1 + n2

    # xt blocked [p(h%128), o(h//128), c] so a 2-h-tile SBUF tile is a
    # plain slice
    xt = nc.dram_tensor("xt", [P, HT, Ctot], mdt, kind="ExternalInput").ap()
    # w1t/v1t pre-blocked: [e, ft, p(h%128), o(h//128), f] so each (e, ft)
    # slice is contiguous and DMAs as 128 x 2KB descriptors
    w1t = nc.dram_tensor("w1t", [E_LOC, FT, P, HT, P], mdt,
                         kind="ExternalInput").ap()
    v1t = nc.dram_tensor("v1t", [E_LOC, FT, P, HT, P], mdt,
                         kind="ExternalInput").ap()
    w2 = nc.dram_tensor("w2", [E_LOC, F, H], mdt, kind="ExternalInput").ap()
    # y transposed [H, Ctot]: GEMM3 keeps w2 stationary (output partition
    # = h-tile), so token columns need no 128-padding
    y = nc.dram_tensor("y", [H, Ctot], dt, kind="ExternalOutput").ap()

    with tile.TileContext(nc) as tc:
        with ExitStack() as ctx:
            xt_pool = ctx.enter_context(tc.tile_pool(name="xt", bufs=HT))
            wst_pool = ctx.enter_context(tc.tile_pool(name="wst", bufs=4))
            w2_pool = ctx.enter_context(tc.tile_pool(name="w2sb", bufs=FT))
            hact_pool = ctx.enter_context(tc.tile_pool(name="hact",
                                                       bufs=2 * FT))
            silu_pool = ctx.enter_context(tc.tile_pool(name="silu", bufs=4))
            out_pool = ctx.enter_context(tc.tile_pool(name="out", bufs=4))
            warm_pool = ctx.enter_context(tc.tile_pool(name="warm", bufs=1))
            # 6 banks for GEMM1/2 g/u accumulators, 2 for GEMM3 so the
            # down accumulators never wait on the GLU drain
            ps_pool = ctx.enter_context(tc.tile_pool(name="ps", bufs=6,
                                                     space="PSUM"))
            psd_pool = ctx.enter_context(tc.tile_pool(name="psd", bufs=2,
                                                      space="PSUM"))

            # Dummy matmuls: PE activity during the DMA head releases the
            # HAM clock gate (4/8 -> 8/8) before the real stream starts.
            warm_sb = warm_pool.tile([P, P], mdt, tag="warm")
            nc.any.memset(warm_sb[:], 0)
            wps = psd_pool.tile([P, 512], dt, tag="psd", name="warm")
            for _ in range(WARM_MMS):
                nc.tensor.matmul(wps[:, :P], warm_sb[:], warm_sb[:],
                                 start=True, stop=True)

            for e in range(E_LOC):
                cnt = n1 if e == 0 else n2
                cbase = 0 if e == 0 else n1
                w2_sb = []

                for ci, (c0, S_) in enumerate(_chunks(cnt)):
                    parts = _parts(S_)

                    # DMA order w1(ft0), xt(ht0-1), v1(ft0), xt(rest).
                    # xt tiles carry two h-tiles per dma_start: descriptor
                    # issue costs ~640ns each on the sync queue, so fewer,
                    # larger issues shorten the head
                    w1s0 = wst_pool.tile([P, HT, P], mdt, tag="wst")
                    v1s0 = wst_pool.tile([P, HT, P], mdt, tag="wst")
                    nc.sync.dma_start(w1s0[:], w1t[e, 0])
                    xt_sb = []
                    for hp in range(HT // 2):
                        t = xt_pool.tile([P, 2, CH], mdt, tag="xt")
                        nc.sync.dma_start(
                            t[:, :, :S_],
                            xt[:, 2 * hp:2 * hp + 2,
                               cbase + c0:cbase + c0 + S_])
                        xt_sb.append(t)
                        if hp == 0:
                            nc.sync.dma_start(v1s0[:], v1t[e, 0])

                    # GEMM1/2 + GLU -> hact_T tiles [128, S_] per f-tile
                    hact_sb = []
                    for ft in range(FT):
                        if ft == 0:
                            w1s, v1s = w1s0, v1s0
                        else:
                            w1s = wst_pool.tile([P, HT, P], mdt, tag="wst")
                            v1s = wst_pool.tile([P, HT, P], mdt, tag="wst")
                            nc.sync.dma_start(w1s[:], w1t[e, ft])
                            nc.sync.dma_start(v1s[:], v1t[e, ft])
                        h_t = hact_pool.tile([P, CH], mdt, tag="hact")
                        g_tiles = [ps_pool.tile([P, 512], dt, tag="ps",
                                                name=f"g{i_}")
                                   for i_ in range(len(parts))]
                        u_tiles = [ps_pool.tile([P, 512], dt, tag="ps",
                                                name=f"u{i_}")
                                   for i_ in range(len(parts))]  # <=4 banks
                        for ht in range(HT):
                            xs = xt_sb[ht // 2]
                            for i_, (o_, p_) in enumerate(parts):
                                nc.tensor.matmul(
                                    g_tiles[i_][:, :p_], w1s[:, ht, :],
                                    xs[:, ht % 2, o_:o_ + p_],
                                    start=(ht == 0), stop=(ht == HT - 1))
                            for i_, (o_, p_) in enumerate(parts):
                                nc.tensor.matmul(
                                    u_tiles[i_][:, :p_], v1s[:, ht, :],
                                    xs[:, ht % 2, o_:o_ + p_],
                                    start=(ht == 0), stop=(ht == HT - 1))
                        for i_, (o_, p_) in enumerate(parts):
                            sl = silu_pool.tile([P, 512], mdt, tag="sl")
                            nc.scalar.activation(sl[:, :p_],
                                                 g_tiles[i_][:, :p_], SILU)
                            nc.vector.tensor_mul(
                                h_t[:, o_:o_ + p_], sl[:, :p_],
                                u_tiles[i_][:, :p_])
                        hact_sb.append(h_t)

                    if ci == 0:
                        for ft in range(FT):
                            t = w2_pool.tile([P, H], mdt, tag="w2",
                                             name=f"w2_{ft}")
                            nc.sync.dma_start(
                                t[:], w2[e, ft * P:(ft + 1) * P, :])
                            w2_sb.append(t)

                    # GEMM3: down^T[h, c] accumulated over f-tiles with w2
                    # stationary (LDW sources long-resident weights, token
                    # columns exact).  Two (h-tile, part) groups interleave
                    # so consecutive matmuls hit different PSUM banks.
                    groups = [(hht, o_, p_) for hht in range(HT)
                              for (o_, p_) in parts]
                    # in the very last chunk there is no following GEMM1/2
                    # work to hide the pair-boundary copy latency, so draw
                    # accumulators from the (by now idle) 6-buffer pool,
                    # and order pairs so the smallest parts finish last
                    last = (e == E_LOC - 1) and (c0 + S_ == cnt)
                    dpool, dtag = (ps_pool, "ps") if last else (psd_pool,
                                                                "psd")
                    if last:
                        groups.sort(key=lambda g: -g[2])
                    for gi in range(0, len(groups), 2):
                        ga, gb = groups[gi], groups[gi + 1]
                        da = dpool.tile([P, 512], dt, tag=dtag, name="da")
                        db = dpool.tile([P, 512], dt, tag=dtag, name="db")
                        for ft in range(FT):
                            for (hht, o_, p_), dd in ((ga, da), (gb, db)):
                                nc.tensor.matmul(
                                    dd[:, :p_],
                                    w2_sb[ft][:, hht * P:(hht + 1) * P],
                                    hact_sb[ft][:, o_:o_ + p_],
                                    start=(ft == 0), stop=(ft == FT - 1))
                        for (hht, o_, p_), dd in ((ga, da), (gb, db)):
                            o_t = out_pool.tile([P, 512], dt, tag="o")
                            nc.any.tensor_copy(o_t[:, :p_], dd[:, :p_])
                            nc.sync.dma_start(
                                y[hht * P:(hht + 1) * P,
                                  cbase + c0 + o_:cbase + c0 + o_ + p_],
                                o_t[:, :p_])
    nc.compile()
    return nc


def _get_nc(n1, n2):
    key = (n1, n2, MM_DTYPE)
    if key not in _nc_cache:
        _nc_cache[key] = _build_nc(n1, n2)
    return _nc_cache[key]


def prepare(x, top_weights, top_experts, w1, v1, w2):
    """Host-side routing + sharded input construction.
    Returns (n1, n2, in_maps, assign, idx, counts, cw)."""
    x = np.asarray(x, dtype=np.float32)
    top_weights = np.asarray(top_weights, dtype=np.float32)
    top_experts = np.asarray(top_experts).astype(np.int64)
    w1 = np.asarray(w1, dtype=np.float32)
    v1 = np.asarray(v1, dtype=np.float32)
    w2 = np.asarray(w2, dtype=np.float32)
    hdt = {"fp16": np.float16, "fp32r": np.float32}.get(MM_DTYPE)
    if hdt is None:
        import ml_dtypes
        hdt = ml_dtypes.bfloat16

    xf = x.reshape(T, H)

    # combine weights per (token, expert); duplicate slots sum
    cw = np.zeros((T, E), dtype=np.float32)
    np.add.at(cw, (np.arange(T)[:, None], top_experts), top_weights)

    idx = [np.nonzero(cw[:, e])[0] for e in range(E)]
    counts = np.array([len(i) for i in idx])

    # slot A = 8 largest experts, slot B = 8 smallest; program built for
    # the max count in each slot -> minimal uniform per-core token count
    order = np.argsort(-counts, kind="stable")
    slot_a, slot_b = order[:N_CORES], order[N_CORES:]
    n1 = max(128, int(counts[slot_a].max()))
    n2 = max(128, int(counts[slot_b].max()))
    assign = [(int(slot_a[m]), int(slot_b[m])) for m in range(N_CORES)]

    def _block(w, ids):
        # [e, F, H] -> [e, ft, p(h%128), o(h//128), f]: each (e, ft)
        # slice contiguous so the DMA runs 128 x 2KB descriptors
        wl = w[ids].reshape(E_LOC, FT, P, HT, P)  # [e, ft, f, o, p]
        return np.ascontiguousarray(
            wl.transpose(0, 1, 4, 3, 2)).astype(hdt)

    in_maps = []
    for m in range(N_CORES):
        ea, eb = assign[m]
        XT = np.zeros((H, n1 + n2), dtype=hdt)
        XT[:, :counts[ea]] = xf[idx[ea]].T.astype(hdt)
        XT[:, n1:n1 + counts[eb]] = xf[idx[eb]].T.astype(hdt)
        # blocked [p(h%128), o(h//128), c]
        XT = np.ascontiguousarray(
            XT.reshape(HT, P, n1 + n2).transpose(1, 0, 2))
        ids = [ea, eb]
        in_maps.append({
            "xt": XT,
            "w1t": _block(w1, ids),
            "v1t": _block(v1, ids),
            "w2": np.ascontiguousarray(w2[ids]).astype(hdt),
        })
    return n1, n2, in_maps, assign, idx, counts, cw


def combine(results, n1, assign, idx, counts, cw):
    """Weighted scatter-add of per-core expert outputs into [B, S, H]."""
    out = np.zeros((T, H), dtype=np.float32)
    for m in range(N_CORES):
        ym = results[m]["y"]  # [H, n1+n2]
        ea, eb = assign[m]
        out[idx[ea]] += ym[:, :counts[ea]].T * cw[idx[ea], ea][:, None]
        out[idx[eb]] += (ym[:, n1:n1 + counts[eb]].T
                         * cw[idx[eb], eb][:, None])
    return out.reshape(B, S, H)


def kernel(x, weights, top_weights, top_experts, w1, v1, w2):
    global LAST_RESULT
    n1, n2, in_maps, assign, idx, counts, cw = prepare(
        x, top_weights, top_experts, w1, v1, w2)
    nc = _get_nc(n1, n2)
    from concourse.bass_utils import run_bass_kernel_spmd
    res = run_bass_kernel_spmd(nc, in_maps, list(range(N_CORES)), trace=TRACE,
                               trace_cores=TRACE_CORES if TRACE else None)
    LAST_RESULT = res
    return combine(res.results, n1, assign, idx, counts, cw)


# revision 29
# speedup vs baseline: 1.0440x; 1.0024x over previous
"""MoE (DbrxExperts) expert-parallel Trainium2 kernel.

Strategy (v2):
  - Host: compute per-(expert,token) combine weights cw, gather each
    expert's routed tokens exactly (no common-C padding), pre-transpose
    operands, fp16 everywhere (rel err ~6e-4 vs 2e-2 budget).
  - Expert->core assignment: the 8 largest experts form "slot A" (one
    per core), the 8 smallest form "slot B".  The SPMD program is built
    for (n1, n2) = (max A count, max B count); this minimizes the
    uniform per-core token count  n1+n2  (4096 -> ~3785 on typical
    routing), which is what the PE stream time scales with.
  - Device (8 cores, SPMD, 2 experts/core): per expert
        gate_T = W1T_blocks^T @ XT     [F, C]   (contract H)
        up_T   = V1T_blocks^T @ XT     [F, C]
        hact_T = silu(gate_T) * up_T   [F, C]   (ACT + DVE, fp16)
        down   = hact_T_blocks^T @ W2  [C, H]   (contract F)
    PSUM fp32, output y fp32.
  - Head optimizations: first f-tile weights DMA'd before the x chunk,
    and a short burst of dummy matmuls warms the PE HAM clock gate
    while the first DMAs land.
  - Host: out[tokens_e] += down_e * cw_e.
"""

import numpy as np
from contextlib import ExitStack

N_CORES = 8
B, S, H = 4, 2048, 1024
F, E = 2048, 16
T = B * S
E_LOC = E // N_CORES  # 2 experts per core (slot A + slot B)

P = 128
HT = H // P   # 8  h-tiles
FT = F // P   # 16 f-tiles
CH = 1024     # max token-chunk width

TRACE = False          # test.py sets this for profiled runs
TRACE_CORES = [7]      # core-0 NTFF capture crashes fast kernels here
MM_DTYPE = "fp16"      # "fp16" | "bf16" | "fp32r"
WARM_MMS = 36          # dummy matmuls to release the HAM clock gate
LAST_RESULT = None     # BassKernelResults of last run (for test.py)

_nc_cache = {}


def _chunks(n):
    """Token chunks of <=CH: [CH, CH, ..., remainder]."""
    out = []
    c0 = 0
    while n - c0 > CH:
        out.append((c0, CH))
        c0 += CH
    out.append((c0, n - c0))
    return out


def _parts(S_):
    """Split a chunk into <=512-wide matmul parts."""
    out = []
    o = 0
    while S_ - o > 512:
        out.append((o, 512))
        o += 512
    out.append((o, S_ - o))
    return out


def _build_nc(n1, n2):
    # NOTE: reads module-global MM_DTYPE
    import concourse.tile as tile
    from concourse import bacc, mybir

    nc = bacc.Bacc("TRN2", target_bir_lowering=False, debug=False,
                   enable_asserts=False, num_devices=N_CORES)
    dt = mybir.dt.float32
    mdt = {"fp16": mybir.dt.float16, "bf16": mybir.dt.bfloat16,
           "fp32r": mybir.dt.float32r}[MM_DTYPE]
    SILU = mybir.ActivationFunctionType.Silu
    Ctot = n1 + n2

    # xt blocked [p(h%128), o(h//128), c] so a 2-h-tile SBUF tile is a
    # plain slice
    xt = nc.dram_tensor("xt", [P, HT, Ctot], mdt, kind="ExternalInput").ap()
    # w1t/v1t pre-blocked: [e, ft, p(h%128), o(h//128), f] so each (e, ft)
    # slice is contiguous and DMAs as 128 x 2KB descriptors
    w1t = nc.dram_tensor("w1t", [E_LOC, FT, P, HT, P], mdt,
                         kind="ExternalInput").ap()
    v1t = nc.dram_tensor("v1t", [E_LOC, FT, P, HT, P], mdt,
                         kind="ExternalInput").ap()
    w2 = nc.dram_tensor("w2", [E_LOC, F, H], mdt, kind="ExternalInput").ap()
    # y transposed [H, Ctot]: GEMM3 keeps w2 stationary (output partition
    # = h-tile), so token columns need no 128-padding
    y = nc.dram_tensor("y", [H, Ctot], dt, kind="ExternalOutput").ap()

    with tile.TileContext(nc) as tc:
        with ExitStack() as ctx:
            xt_pool = ctx.enter_context(tc.tile_pool(name="xt", bufs=HT))
            wst_pool = ctx.enter_context(tc.tile_pool(name="wst", bufs=4))
            w2_pool = ctx.enter_context(tc.tile_pool(name="w2sb", bufs=FT))
            hact_pool = ctx.enter_context(tc.tile_pool(name="hact",
                                                       bufs=2 * FT))
            silu_pool = ctx.enter_context(tc.tile_pool(name="silu", bufs=4))
            out_pool = ctx.enter_context(tc.tile_pool(name="out", bufs=4))
            warm_pool = ctx.enter_context(tc.tile_pool(name="warm", bufs=1))
            # 6 banks for GEMM1/2 g/u accumulators, 2 for GEMM3 so the
            # down accumulators never wait on the GLU drain
            ps_pool = ctx.enter_context(tc.tile_pool(name="ps", bufs=6,
                                                     space="PSUM"))
            psd_pool = ctx.enter_context(tc.tile_pool(name="psd", bufs=2,
                                                      space="PSUM"))

            # Dummy matmuls: PE activity during the DMA head releases the
            # HAM clock gate (4/8 -> 8/8) before the real stream starts.
            warm_sb = warm_pool.tile([P, P], mdt, tag="warm")
            nc.any.memset(warm_sb[:], 0)
            wps = psd_pool.tile([P, 512], dt, tag="psd", name="warm")
            for _ in range(WARM_MMS):
                nc.tensor.matmul(wps[:, :P], warm_sb[:], warm_sb[:],
                                 start=True, stop=True)

            for e in range(E_LOC):
                cnt = n1 if e == 0 else n2
                cbase = 0 if e == 0 else n1
                w2_sb = []

                for ci, (c0, S_) in enumerate(_chunks(cnt)):
                    parts = _parts(S_)

                    # DMA order w1(ft0), xt(ht0-1), v1(ft0), xt(rest).
                    # xt tiles carry two h-tiles per dma_start: descriptor
                    # issue costs ~640ns each on the sync queue, so fewer,
                    # larger issues shorten the head
                    w1s0 = wst_pool.tile([P, HT, P], mdt, tag="wst")
                    v1s0 = wst_pool.tile([P, HT, P], mdt, tag="wst")
                    nc.sync.dma_start(w1s0[:], w1t[e, 0])
                    xt_sb = []
                    for hp in range(HT // 2):
                        t = xt_pool.tile([P, 2, CH], mdt, tag="xt")
                        nc.sync.dma_start(
                            t[:, :, :S_],
                            xt[:, 2 * hp:2 * hp + 2,
                               cbase + c0:cbase + c0 + S_])
                        xt_sb.append(t)
                        if hp == 0:
                            nc.sync.dma_start(v1s0[:], v1t[e, 0])

                    # GEMM1/2 + GLU -> hact_T tiles [128, S_] per f-tile
                    hact_sb = []
                    for ft in range(FT):
                        if ft == 0:
                            w1s, v1s = w1s0, v1s0
                        else:
                            w1s = wst_pool.tile([P, HT, P], mdt, tag="wst")
                            v1s = wst_pool.tile([P, HT, P], mdt, tag="wst")
                            nc.sync.dma_start(w1s[:], w1t[e, ft])
                            nc.sync.dma_start(v1s[:], v1t[e, ft])
                        h_t = hact_pool.tile([P, CH], mdt, tag="hact")
                        g_tiles = [ps_pool.tile([P, 512], dt, tag="ps",
                                                name=f"g{i_}")
                                   for i_ in range(len(parts))]
                        u_tiles = [ps_pool.tile([P, 512], dt, tag="ps",
                                                name=f"u{i_}")
                                   for i_ in range(len(parts))]  # <=4 banks
                        for ht in range(HT):
                            xs = xt_sb[ht // 2]
                            for i_, (o_, p_) in enumerate(parts):
                                nc.tensor.matmul(
                                    g_tiles[i_][:, :p_], w1s[:, ht, :],
                                    xs[:, ht % 2, o_:o_ + p_],
                                    start=(ht == 0), stop=(ht == HT - 1))
                            for i_, (o_, p_) in enumerate(parts):
                                nc.tensor.matmul(
                                    u_tiles[i_][:, :p_], v1s[:, ht, :],
                                    xs[:, ht % 2, o_:o_ + p_],
                                    start=(ht == 0), stop=(ht == HT - 1))
                        for i_, (o_, p_) in enumerate(parts):
                            sl = silu_pool.tile([P, 512], mdt, tag="sl")
                            nc.scalar.activation(sl[:, :p_],
                                                 g_tiles[i_][:, :p_], SILU)
                            nc.vector.tensor_mul(
                                h_t[:, o_:o_ + p_], sl[:, :p_],
                                u_tiles[i_][:, :p_])
                        hact_sb.append(h_t)

                    if ci == 0:
                        for ft in range(FT):
                            t = w2_pool.tile([P, H], mdt, tag="w2",
                                             name=f"w2_{ft}")
                            nc.sync.dma_start(
                                t[:], w2[e, ft * P:(ft + 1) * P, :])
                            w2_sb.append(t)

                    # GEMM3: down^T[h, c] accumulated over f-tiles with w2
                    # stationary (LDW sources long-resident weights, token
                    # columns exact).  Two (h-tile, part) groups interleave
                    # so consecutive matmuls hit different PSUM banks.
                    groups = [(hht, o_, p_) for hht in range(HT)
                              for (o_, p_) in parts]
                    # in the very last chunk there is no following GEMM1/2
                    # work to hide the pair-boundary copy latency, so draw
                    # accumulators from the (by now idle) 6-buffer pool
                    last = (e == E_LOC - 1) and (c0 + S_ == cnt)
                    dpool, dtag = (ps_pool, "ps") if last else (psd_pool,
                                                                "psd")
                    for gi in range(0, len(groups), 2):
                        ga, gb = groups[gi], groups[gi + 1]
                        da = dpool.tile([P, 512], dt, tag=dtag, name="da")
                        db = dpool.tile([P, 512], dt, tag=dtag, name="db")
                        for ft in range(FT):
                            for (hht, o_, p_), dd in ((ga, da), (gb, db)):
                                nc.tensor.matmul(
                                    dd[:, :p_],
                                    w2_sb[ft][:, hht * P:(hht + 1) * P],
                                    hact_sb[ft][:, o_:o_ + p_],
                                    start=(ft == 0), stop=(ft == FT - 1))
                        for (hht, o_, p_), dd in ((ga, da), (gb, db)):
                            o_t = out_pool.tile([P, 512], dt, tag="o")
                            nc.any.tensor_copy(o_t[:, :p_], dd[:, :p_])
                            nc.sync.dma_start(
                                y[hht * P:(hht + 1) * P,
                                  cbase + c0 + o_:cbase + c0 + o_ + p_],
                                o_t[:, :p_])
    nc.compile()
    return nc


def _get_nc(n1, n2):
    key = (n1, n2, MM_DTYPE)
    if key not in _nc_cache:
        _nc_cache[key] = _build_nc(n1, n2)
    return _nc_cache[key]


def prepare(x, top_weights, top_experts, w1, v1, w2):
    """Host-side routing + sharded input construction.
    Returns (n1, n2, in_maps, assign, idx, counts, cw)."""
    x = np.asarray(x, dtype=np.float32)
    top_weights = np.asarray(top_weights, dtype=np.float32)
    top_experts = np.asarray(top_experts).astype(np.int64)
    w1 = np.asarray(w1, dtype=np.float32)
    v1 = np.asarray(v1, dtype=np.float32)
    w2 = np.asarray(w2, dtype=np.float32)
    hdt = {"fp16": np.float16, "fp32r": np.float32}.get(MM_DTYPE)
    if hdt is None:
        import ml_dtypes
        hdt = ml_dtypes.bfloat16

    xf = x.reshape(T, H)

    # combine weights per (token, expert); duplicate slots sum
    cw = np.zeros((T, E), dtype=np.float32)
    np.add.at(cw, (np.arange(T)[:, None], top_experts), top_weights)

    idx = [np.nonzero(cw[:, e])[0] for e in range(E)]
    counts = np.array([len(i) for i in idx])

    # slot A = 8 largest experts, slot B = 8 smallest; program built for
    # the max count in each slot -> minimal uniform per-core token count
    order = np.argsort(-counts, kind="stable")
    slot_a, slot_b = order[:N_CORES], order[N_CORES:]
    n1 = max(128, int(counts[slot_a].max()))
    n2 = max(128, int(counts[slot_b].max()))
    assign = [(int(slot_a[m]), int(slot_b[m])) for m in range(N_CORES)]

    def _block(w, ids):
        # [e, F, H] -> [e, ft, p(h%128), o(h//128), f]: each (e, ft)
        # slice contiguous so the DMA runs 128 x 2KB descriptors
        wl = w[ids].reshape(E_LOC, FT, P, HT, P)  # [e, ft, f, o, p]
        return np.ascontiguousarray(
            wl.transpose(0, 1, 4, 3, 2)).astype(hdt)

    in_maps = []
    for m in range(N_CORES):
        ea, eb = assign[m]
        XT = np.zeros((H, n1 + n2), dtype=hdt)
        XT[:, :counts[ea]] = xf[idx[ea]].T.astype(hdt)
        XT[:, n1:n1 + counts[eb]] = xf[idx[eb]].T.astype(hdt)
        # blocked [p(h%128), o(h//128), c]
        XT = np.ascontiguousarray(
            XT.reshape(HT, P, n1 + n2).transpose(1, 0, 2))
        ids = [ea, eb]
        in_maps.append({
            "xt": XT,
            "w1t": _block(w1, ids),
            "v1t": _block(v1, ids),
            "w2": np.ascontiguousarray(w2[ids]).astype(hdt),
        })
    return n1, n2, in_maps, assign, idx, counts, cw


def combine(results, n1, assign, idx, counts, cw):
    """Weighted scatter-add of per-core expert outputs into [B, S, H]."""
    out = np.zeros((T, H), dtype=np.float32)
    for m in range(N_CORES):
        ym = results[m]["y"]  # [H, n1+n2]
        ea, eb = assign[m]
        out[idx[ea]] += ym[:, :counts[ea]].T * cw[idx[ea], ea][:, None]
        out[idx[eb]] += (ym[:, n1:n1 + counts[eb]].T
                         * cw[idx[eb], eb][:, None])
    return out.reshape(B, S, H)


def kernel(x, weights, top_weights, top_experts, w1, v1, w2):
    global LAST_RESULT
    n1, n2, in_maps, assign, idx, counts, cw = prepare(
        x, top_weights, top_experts, w1, v1, w2)
    nc = _get_nc(n1, n2)
    from concourse.bass_utils import run_bass_kernel_spmd
    res = run_bass_kernel_spmd(nc, in_maps, list(range(N_CORES)), trace=TRACE,
                               trace_cores=TRACE_CORES if TRACE else None)
    LAST_RESULT = res
    return combine(res.results, n1, assign, idx, counts, cw)
